# revision 16
# baseline (speedup 1.0000x reference)
"""Trainium2 Bass kernel for nn_Attention (general-score attention with
masked softmax), data-parallel over batch across 8 NeuronCores.

Math (per batch), matching the reference exactly for {0,1} float masks:
    raw[t,s]  = sum_e (hidden @ W)[t,e] * enc[s,e]       (associativity trick:
                (hidden @ W) @ enc^T  ==  hidden @ (enc @ W^T)^T, saves 25%
                FLOPs and avoids materializing proj)
    attn_energies = raw * mask            (mask in {0,1} so mask^2 == mask)
    e = exp(x - max_s x) * mask
    attn = e / (sum_s e + 1e-6)
    context = attn @ enc_value

v3, from HAM/trace analysis of v1 (156us) and v2 (128us):
  - All three gemms in fp16 with f32 PSUM accumulation (measured end-to-end
    rel err ~2.4e-3, same as all-f32r).  Warm PE issue rate is ~259ns per
    512-wide 16-bit matmul; the PE clock-gates to 1.2GHz after any ~3.4us
    idle window, so the whole kernel is one gap-free PE instruction chain.
  - mm1 fuses BOTH batches into one moving operand and consumes
    (w[dt], hidT01[dt]) DMA pairs as they land dt-outer; the last two dt
    rounds go et-wise with the PSUM->SBUF drain emitted right after each
    et's stop so the next pass never waits on a drain (v2 lost ~4us there).
  - 8 junk transposes of the identity warm the HAM clock gate during the
    ~9us DMA/preamble dead time, so mm1 runs at 2.4GHz almost from the start.
  - attnT copies go on GpSimd (idle engine) - in v2 they queued behind the
    Scalar softmax backlog at the B->C boundary (~1.7us stall).
  - ae+aw are packed into one [128,2048] tile and one DMA per tile; encT
    and val are marshaled partition-major on the host so each loads with 2
    DMAs per batch.  The Sync engine issues each dma_start serially at
    ~0.73us, so fewer+bigger transfers keep it off the critical path.
  - the final tile's ctx drain is split across DVE+Scalar with two DMA
    halves to shorten the end-of-kernel tail.
"""
import os

import numpy as np

B, TRG, SRC, ENCD, TRGD = 16, 512, 1024, 1024, 1024
NCORES = 8
BPC = B // NCORES  # batches per core
P = 128
nD = TRGD // P   # 8 contraction tiles over d
nE = ENCD // P   # 8 over e
nS = SRC // P    # 8 over s
nT = TRG // P    # 4 t-tiles per batch
TRG2 = BPC * TRG  # both batches fused along t: 1024

_cache = {}

LAST_EXEC_NS = None
LAST_RESULTS = None


def _build():
    import bass_rust
    import concourse.mybir as mybir
    import concourse.tile as tile
    from concourse import bacc
    from concourse.masks import make_identity

    _add_dep = bass_rust.add_dep_helper

    F32 = mybir.dt.float32
    FP16 = mybir.dt.float16
    ALU = mybir.AluOpType
    AXL = mybir.AxisListType
    ACT_EXP = mybir.ActivationFunctionType.Exp

    nc = bacc.Bacc("TRN2", target_bir_lowering=False, debug=False)

    hidT_d = nc.dram_tensor("hidT", (TRGD, TRG2), FP16, kind="ExternalInput")
    w_d = nc.dram_tensor("w", (TRGD, ENCD), FP16, kind="ExternalInput")
    encT_d = nc.dram_tensor("encT", (BPC, P, nE, SRC), FP16,
                            kind="ExternalInput")
    val_d = nc.dram_tensor("val", (BPC, P, nS, TRGD), FP16,
                           kind="ExternalInput")
    mask_d = nc.dram_tensor("mask", (BPC, 1, SRC), F32, kind="ExternalInput")
    aeaw_d = nc.dram_tensor("aeaw", (BPC, TRG, 2 * SRC), FP16,
                            kind="ExternalOutput")
    ctx_d = nc.dram_tensor("ctx", (BPC, TRG, TRGD), FP16,
                           kind="ExternalOutput")

    with tile.TileContext(nc) as tc:
        with (
            tc.tile_pool(name="const", bufs=1) as const,
            tc.tile_pool(name="wp", bufs=1) as wp,
            tc.tile_pool(name="big", bufs=1) as big,
            tc.tile_pool(name="sm", bufs=2) as sm,
            tc.tile_pool(name="ps", bufs=4, space="PSUM") as psp,
        ):
            ident = const.tile([P, P], F32)
            make_identity(nc, ident[:])
            identh = const.tile([P, P], FP16)
            nc.vector.tensor_copy(identh[:], ident[:])

            # PE program order is pinned with an explicit linear chain so the
            # scheduler can never interleave accumulation groups or delay a
            # group's stop.
            pe_prev = [None]

            def chain(mm):
                if pe_prev[0] is not None:
                    _add_dep(mm.ins, pe_prev[0].ins, sync=False,
                             reason="pe order")
                pe_prev[0] = mm
                return mm

            # ---- loads (issue order == consumption order) ----
            w_sb = [wp.tile([P, ENCD], FP16, tag=f"w{i}", name=f"w_sb{i}")
                    for i in range(nD)]
            hidT_sb = [big.tile([P, TRG2], FP16, tag=f"hidT{i}",
                                name=f"hidT_sb{i}") for i in range(nD)]
            for i in range(nD):
                nc.sync.dma_start(out=w_sb[i][:], in_=w_d[i * P:(i + 1) * P, :])
                nc.sync.dma_start(out=hidT_sb[i][:],
                                  in_=hidT_d[i * P:(i + 1) * P, :])
            maskbs = []
            for b in range(BPC):
                maskb = sm.tile([P, SRC], F32, tag="maskb", name=f"maskb{b}")
                nc.sync.dma_start(out=maskb[:],
                                  in_=mask_d[b].to_broadcast((P, SRC)))
                maskb_hf = sm.tile([P, SRC], FP16, tag="maskb_hf",
                                   name=f"maskb_hf{b}")
                nc.vector.tensor_copy(maskb_hf[:], maskb[:])
                maskbs.append((maskb, maskb_hf))
            encT_sb = []
            val_sb = []
            for b in range(BPC):
                e_t = big.tile([P, nE, SRC], FP16, tag="encT", bufs=2,
                               name=f"encT_sb{b}")
                for g in range(2):
                    gs = slice(g * (nE // 2), (g + 1) * (nE // 2))
                    nc.sync.dma_start(out=e_t[:, gs, :],
                                      in_=encT_d[b, :, gs, :])
                v_t = big.tile([P, nS, TRGD], FP16, tag="val", bufs=2,
                               name=f"val_sb{b}")
                for g in range(2):
                    gs = slice(g * (nS // 2), (g + 1) * (nS // 2))
                    nc.sync.dma_start(out=v_t[:, gs, :],
                                      in_=val_d[b, :, gs, :])
                encT_sb.append(e_t)
                val_sb.append(v_t)

            # ---- mm1: HpT[e, t01] = sum_d W[d,e] * hidT01[d, t01] ----
            # two half-passes of 4 et each (4 psum bufs per pass).  dt-outer
            # for DMA pair-wise consumption, but the last two dt rounds go
            # et-wise with the drain right after each stop so ring slots free
            # up staggered instead of all at the end.
            HpT = big.tile([P, nE, TRG2], FP16, tag="HpT", name="HpT")
            drain_eng = [0]

            def drain(dst, src):
                if drain_eng[0] % 2 == 0:
                    nc.vector.tensor_copy(dst, src)
                else:
                    nc.scalar.copy(dst, src)
                drain_eng[0] += 1

            def mm1_mm(pp, dt, et):
                for h in range(2):
                    hs = slice(h * 512, (h + 1) * 512)
                    chain(nc.tensor.matmul(
                        pp[:, hs], w_sb[dt][:, et * P:(et + 1) * P],
                        hidT_sb[dt][:, hs],
                        start=(dt == 0), stop=(dt == nD - 1)))

            def emit_mm1_pass(ets, warm=False):
                pps = [psp.tile([P, TRG2], F32, tag="ps", name=f"mm1ps{et}")
                       for et in ets]
                if warm:
                    # junk transposes of the identity: keep the PE busy
                    # during the DMA/preamble dead time so the HAM clock
                    # gate is warm (2.4GHz) when real work arrives.  The
                    # garbage psum is overwritten by mm1's start=True.
                    junk_view = pps[0][:].bitcast(FP16)
                    for _ in range(24):
                        chain(nc.tensor.matmul(
                            junk_view[:, 0:P], identh[:], identh[:],
                            is_transpose=True, skip_group_check=True))
                for dt in range(nD - 2):
                    for i, et in enumerate(ets):
                        mm1_mm(pps[i], dt, et)
                for i, et in enumerate(ets):
                    for dt in (nD - 2, nD - 1):
                        mm1_mm(pps[i], dt, et)
                    drain(HpT[:, et, :], pps[i][:])

            emit_mm1_pass(range(0, nE // 2), warm=True)
            emit_mm1_pass(range(nE // 2, nE))

            # ---- mm2 + masked softmax over 8 supertiles (b, tt) ----
            tiles = [(b, tt) for b in range(BPC) for tt in range(nT)]
            pks = []
            attnTs = {}

            def emit_mm2(k):
                b, tt = tiles[k]
                ts = slice(b * TRG + tt * P, b * TRG + (tt + 1) * P)
                en_ps = psp.tile([P, SRC], F32, tag="ps", name=f"en{b}{tt}")
                for et in range(nE):
                    for h in range(2):
                        hs = slice(h * 512, (h + 1) * 512)
                        chain(nc.tensor.matmul(en_ps[:, hs], HpT[:, et, ts],
                                               encT_sb[b][:, et, hs],
                                               start=(et == 0),
                                               stop=(et == nE - 1)))
                return en_ps

            def emit_softmax(k, en_ps):
                b, tt = tiles[k]
                maskb, maskb_hf = maskbs[b]
                x = sm.tile([P, SRC], F32, tag="x")
                nc.vector.tensor_mul(x[:], en_ps[:], maskb[:])
                # packed [ae | attn] tile: one output DMA per supertile
                pk = sm.tile([P, 2 * SRC], FP16, tag="aeaw", bufs=7,
                             name=f"aeaw{b}{tt}")
                nc.scalar.copy(pk[:, :SRC], x[:])
                negm = sm.tile([P, 1], F32, tag="negm")
                nc.vector.tensor_reduce(negm[:], x[:], axis=AXL.X,
                                        op=ALU.max, negate=True)
                ex = sm.tile([P, SRC], FP16, tag="ex")
                nc.scalar.activation(ex[:], x[:], ACT_EXP, bias=negm[:],
                                     scale=1.0)
                rowsum = sm.tile([P, 1], F32, tag="rowsum")
                nc.vector.scalar_tensor_tensor(ex[:], ex[:], 1.0,
                                               maskb_hf[:],
                                               op0=ALU.mult, op1=ALU.mult,
                                               accum_out=rowsum[:])
                z = sm.tile([P, 1], F32, tag="z")
                nc.vector.tensor_scalar_add(z[:], rowsum[:], 1e-6)
                rz = sm.tile([P, 1], F32, tag="rz")
                nc.vector.reciprocal(rz[:], z[:])
                nc.vector.tensor_scalar_mul(pk[:, SRC:], ex[:], rz[:])
                nc.sync.dma_start(out=aeaw_d[b, tt * P:(tt + 1) * P, :],
                                  in_=pk[:])
                pks.append(pk)
                # transpose attn on the DMA xbar (SBUF->SBUF, ~1.8us/tile)
                # as soon as it exists: frees the PE of 8 transposes per
                # tile and decouples mm3 from any engine-queue backlog
                attnT = sm.tile([P, nS, P], FP16, tag="attnT", bufs=8,
                                name=f"attnT{k}")
                nc.sync.dma_start(out=attnT[:], in_=pk[:, SRC:],
                                  transpose=True)
                attnTs[k] = attnT

            for k in range(len(tiles)):
                en_ps = emit_mm2(k)
                emit_softmax(k, en_ps)

            def emit_mm3(k):
                b, tt = tiles[k]
                attnT = attnTs.pop(k)
                last = (k == len(tiles) - 1)
                ctx_ps = psp.tile([P, TRGD], F32, tag="ps", name=f"ctx{k}")
                for st in range(nS):
                    for h in range(2):
                        hs = slice(h * 512, (h + 1) * 512)
                        chain(nc.tensor.matmul(ctx_ps[:, hs],
                                               attnT[:, st, :],
                                               val_sb[b][:, st, hs],
                                               start=(st == 0),
                                               stop=(st == nS - 1)))
                ctx_sb = sm.tile([P, TRGD], FP16, tag="ctx_sb")
                rows = slice(tt * P, (tt + 1) * P)
                if last:
                    # split the final drain across two engines + two DMAs to
                    # shorten the end-of-kernel tail
                    nc.vector.tensor_copy(ctx_sb[:, :512], ctx_ps[:, :512])
                    nc.scalar.copy(ctx_sb[:, 512:], ctx_ps[:, 512:])
                    nc.sync.dma_start(out=ctx_d[b, rows, 0:512],
                                      in_=ctx_sb[:, :512])
                    nc.sync.dma_start(out=ctx_d[b, rows, 512:],
                                      in_=ctx_sb[:, 512:])
                else:
                    nc.scalar.copy(ctx_sb[:], ctx_ps[:])
                    nc.sync.dma_start(out=ctx_d[b, rows, :], in_=ctx_sb[:])

            for k in range(len(tiles)):
                emit_mm3(k)

    nc.compile()
    return nc


def kernel(hidden, encoder_outputs, encoder_value, encoder_mask, W):
    global LAST_EXEC_NS, LAST_RESULTS
    from concourse.bass_utils import run_bass_kernel_spmd

    if "nc" not in _cache:
        _cache["nc"] = _build()
    nc = _cache["nc"]

    hidden = np.ascontiguousarray(hidden, dtype=np.float32)
    encoder_outputs = np.ascontiguousarray(encoder_outputs, dtype=np.float32)
    encoder_value = np.ascontiguousarray(encoder_value, dtype=np.float32)
    encoder_mask = np.ascontiguousarray(encoder_mask, dtype=np.float32)
    W = np.ascontiguousarray(W, dtype=np.float32)

    w_hf = W.astype(np.float16)
    in_maps = []
    for c in range(NCORES):
        sl = slice(c * BPC, (c + 1) * BPC)
        hid2 = hidden[sl]  # (2, TRG, TRGD)
        hidT01 = np.concatenate([hid2[0].T, hid2[1].T], axis=1)
        # partition-major marshaling: x_d[b, p, tile, free] so each batch
        # loads with 2 contiguous DMAs
        encT = encoder_outputs[sl].transpose(0, 2, 1)  # (BPC, ENCD, SRC)
        encT_pm = encT.reshape(BPC, nE, P, SRC).transpose(0, 2, 1, 3)
        val_pm = encoder_value[sl].reshape(BPC, nS, P, TRGD).transpose(
            0, 2, 1, 3)
        in_maps.append({
            "hidT": np.ascontiguousarray(hidT01.astype(np.float16)),
            "w": w_hf,
            "encT": np.ascontiguousarray(encT_pm.astype(np.float16)),
            "val": np.ascontiguousarray(val_pm.astype(np.float16)),
            "mask": encoder_mask[sl][:, None, :],
        })

    trace = bool(int(os.environ.get("KERNEL_TRACE", "0")))
    res = run_bass_kernel_spmd(nc, in_maps, core_ids=list(range(NCORES)),
                               trace=trace)
    LAST_EXEC_NS = res.exec_time_ns
    LAST_RESULTS = res

    aeaw = [res.results[c]["aeaw"] for c in range(NCORES)]
    context = np.concatenate([res.results[c]["ctx"] for c in range(NCORES)],
                             axis=0).astype(np.float32)
    attn_energies = np.concatenate(
        [a[:, :, :SRC] for a in aeaw], axis=0).astype(np.float32)
    attn_weights = np.concatenate(
        [a[:, :, SRC:] for a in aeaw], axis=0).astype(np.float32)
    return context, attn_weights, attn_energies


# revision 17
# speedup vs baseline: 1.0750x; 1.0750x over previous
"""Trainium2 Bass kernel for nn_Attention (general-score attention with
masked softmax), data-parallel over batch across 8 NeuronCores.

Math (per batch), matching the reference exactly for {0,1} float masks:
    raw[t,s]  = sum_e (hidden @ W)[t,e] * enc[s,e]       (associativity trick:
                (hidden @ W) @ enc^T  ==  hidden @ (enc @ W^T)^T, saves 25%
                FLOPs and avoids materializing proj)
    attn_energies = raw * mask            (mask in {0,1} so mask^2 == mask)
    e = exp(x - max_s x) * mask
    attn = e / (sum_s e + 1e-6)
    context = attn @ enc_value

v3, from HAM/trace analysis of v1 (156us) and v2 (128us):
  - All three gemms in fp16 with f32 PSUM accumulation (measured end-to-end
    rel err ~2.4e-3, same as all-f32r).  Warm PE issue rate is ~259ns per
    512-wide 16-bit matmul; the PE clock-gates to 1.2GHz after any ~3.4us
    idle window, so the whole kernel is one gap-free PE instruction chain.
  - mm1 fuses BOTH batches into one moving operand and consumes
    (w[dt], hidT01[dt]) DMA pairs as they land dt-outer; the last two dt
    rounds go et-wise with the PSUM->SBUF drain emitted right after each
    et's stop so the next pass never waits on a drain (v2 lost ~4us there).
  - 8 junk transposes of the identity warm the HAM clock gate during the
    ~9us DMA/preamble dead time, so mm1 runs at 2.4GHz almost from the start.
  - attnT copies go on GpSimd (idle engine) - in v2 they queued behind the
    Scalar softmax backlog at the B->C boundary (~1.7us stall).
  - ae+aw are packed into one [128,2048] tile and one DMA per tile; encT
    and val are marshaled partition-major on the host so each loads with 2
    DMAs per batch.  The Sync engine issues each dma_start serially at
    ~0.73us, so fewer+bigger transfers keep it off the critical path.
  - the final tile's ctx drain is split across DVE+Scalar with two DMA
    halves to shorten the end-of-kernel tail.
"""
import os

import numpy as np

B, TRG, SRC, ENCD, TRGD = 16, 512, 1024, 1024, 1024
NCORES = 8
BPC = B // NCORES  # batches per core
P = 128
nD = TRGD // P   # 8 contraction tiles over d
nE = ENCD // P   # 8 over e
nS = SRC // P    # 8 over s
nT = TRG // P    # 4 t-tiles per batch
TRG2 = BPC * TRG  # both batches fused along t: 1024

_cache = {}

LAST_EXEC_NS = None
LAST_RESULTS = None


def _build():
    import bass_rust
    import concourse.mybir as mybir
    import concourse.tile as tile
    from concourse import bacc
    from concourse.masks import make_identity

    _add_dep = bass_rust.add_dep_helper

    F32 = mybir.dt.float32
    FP16 = mybir.dt.float16
    ALU = mybir.AluOpType
    AXL = mybir.AxisListType
    ACT_EXP = mybir.ActivationFunctionType.Exp

    nc = bacc.Bacc("TRN2", target_bir_lowering=False, debug=False)

    hidT_d = nc.dram_tensor("hidT", (TRGD, TRG2), FP16, kind="ExternalInput")
    w_d = nc.dram_tensor("w", (TRGD, ENCD), FP16, kind="ExternalInput")
    encT_d = nc.dram_tensor("encT", (BPC, P, nE, SRC), FP16,
                            kind="ExternalInput")
    val_d = nc.dram_tensor("val", (BPC, P, nS, TRGD), FP16,
                           kind="ExternalInput")
    mask_d = nc.dram_tensor("mask", (BPC, 1, SRC), F32, kind="ExternalInput")
    aeaw_d = nc.dram_tensor("aeaw", (BPC, TRG, 2 * SRC), FP16,
                            kind="ExternalOutput")
    ctx_d = nc.dram_tensor("ctx", (BPC, TRG, TRGD), FP16,
                           kind="ExternalOutput")

    with tile.TileContext(nc) as tc:
        with (
            tc.tile_pool(name="const", bufs=1) as const,
            tc.tile_pool(name="wp", bufs=1) as wp,
            tc.tile_pool(name="big", bufs=1) as big,
            tc.tile_pool(name="sm", bufs=2) as sm,
            tc.tile_pool(name="ps", bufs=4, space="PSUM") as psp,
        ):
            ident = const.tile([P, P], F32)
            make_identity(nc, ident[:])
            identh = const.tile([P, P], FP16)
            nc.vector.tensor_copy(identh[:], ident[:])

            # PE program order is pinned with an explicit linear chain so the
            # scheduler can never interleave accumulation groups or delay a
            # group's stop.
            pe_prev = [None]

            def chain(mm):
                if pe_prev[0] is not None:
                    _add_dep(mm.ins, pe_prev[0].ins, sync=False,
                             reason="pe order")
                pe_prev[0] = mm
                return mm

            # ---- loads (issue order == consumption order) ----
            w_sb = [wp.tile([P, ENCD], FP16, tag=f"w{i}", name=f"w_sb{i}")
                    for i in range(nD)]
            hidT_sb = [big.tile([P, TRG2], FP16, tag=f"hidT{i}",
                                name=f"hidT_sb{i}") for i in range(nD)]
            for i in range(nD):
                nc.sync.dma_start(out=w_sb[i][:], in_=w_d[i * P:(i + 1) * P, :])
                nc.sync.dma_start(out=hidT_sb[i][:],
                                  in_=hidT_d[i * P:(i + 1) * P, :])
            maskbs = []
            for b in range(BPC):
                maskb = sm.tile([P, SRC], F32, tag="maskb", name=f"maskb{b}")
                nc.sync.dma_start(out=maskb[:],
                                  in_=mask_d[b].to_broadcast((P, SRC)))
                maskb_hf = sm.tile([P, SRC], FP16, tag="maskb_hf",
                                   name=f"maskb_hf{b}")
                nc.vector.tensor_copy(maskb_hf[:], maskb[:])
                maskbs.append((maskb, maskb_hf))
            encT_sb = []
            val_sb = []
            for b in range(BPC):
                e_t = big.tile([P, nE, SRC], FP16, tag="encT", bufs=2,
                               name=f"encT_sb{b}")
                for g in range(2):
                    gs = slice(g * (nE // 2), (g + 1) * (nE // 2))
                    nc.sync.dma_start(out=e_t[:, gs, :],
                                      in_=encT_d[b, :, gs, :])
                v_t = big.tile([P, nS, TRGD], FP16, tag="val", bufs=2,
                               name=f"val_sb{b}")
                for g in range(2):
                    gs = slice(g * (nS // 2), (g + 1) * (nS // 2))
                    nc.sync.dma_start(out=v_t[:, gs, :],
                                      in_=val_d[b, :, gs, :])
                encT_sb.append(e_t)
                val_sb.append(v_t)

            # ---- mm1: HpT[e, t01] = sum_d W[d,e] * hidT01[d, t01] ----
            # two half-passes of 4 et each (4 psum bufs per pass).  dt-outer
            # for DMA pair-wise consumption, but the last two dt rounds go
            # et-wise with the drain right after each stop so ring slots free
            # up staggered instead of all at the end.
            HpT = big.tile([P, nE, TRG2], FP16, tag="HpT", name="HpT")
            drain_eng = [0]

            def drain(dst, src):
                if drain_eng[0] % 2 == 0:
                    nc.vector.tensor_copy(dst, src)
                else:
                    nc.scalar.copy(dst, src)
                drain_eng[0] += 1

            def mm1_mm(pp, dt, et):
                for h in range(2):
                    hs = slice(h * 512, (h + 1) * 512)
                    chain(nc.tensor.matmul(
                        pp[:, hs], w_sb[dt][:, et * P:(et + 1) * P],
                        hidT_sb[dt][:, hs],
                        start=(dt == 0), stop=(dt == nD - 1)))

            def emit_mm1_pass(ets, warm=False):
                pps = [psp.tile([P, TRG2], F32, tag="ps", name=f"mm1ps{et}")
                       for et in ets]
                if warm:
                    # junk transposes of the identity: keep the PE busy
                    # during the DMA/preamble dead time so the HAM clock
                    # gate is warm (2.4GHz) when real work arrives.  The
                    # garbage psum is overwritten by mm1's start=True.
                    junk_view = pps[0][:].bitcast(FP16)
                    for _ in range(24):
                        chain(nc.tensor.matmul(
                            junk_view[:, 0:P], identh[:], identh[:],
                            is_transpose=True, skip_group_check=True))
                for dt in range(nD - 2):
                    for i, et in enumerate(ets):
                        mm1_mm(pps[i], dt, et)
                for i, et in enumerate(ets):
                    for dt in (nD - 2, nD - 1):
                        mm1_mm(pps[i], dt, et)
                    drain(HpT[:, et, :], pps[i][:])

            emit_mm1_pass(range(0, nE // 2), warm=True)
            emit_mm1_pass(range(nE // 2, nE))

            # ---- mm2 + masked softmax over 8 supertiles (b, tt) ----
            tiles = [(b, tt) for b in range(BPC) for tt in range(nT)]
            pks = []
            attnTs = {}

            def emit_mm2(k):
                b, tt = tiles[k]
                ts = slice(b * TRG + tt * P, b * TRG + (tt + 1) * P)
                en_ps = psp.tile([P, SRC], F32, tag="ps", name=f"en{b}{tt}")
                for et in range(nE):
                    for h in range(2):
                        hs = slice(h * 512, (h + 1) * 512)
                        chain(nc.tensor.matmul(en_ps[:, hs], HpT[:, et, ts],
                                               encT_sb[b][:, et, hs],
                                               start=(et == 0),
                                               stop=(et == nE - 1)))
                return en_ps

            def emit_softmax(k, en_ps):
                b, tt = tiles[k]
                maskb, maskb_hf = maskbs[b]
                x = sm.tile([P, SRC], F32, tag="x")
                nc.vector.tensor_mul(x[:], en_ps[:], maskb[:])
                # packed [ae | attn] tile: one output DMA per supertile
                pk = sm.tile([P, 2 * SRC], FP16, tag="aeaw", bufs=7,
                             name=f"aeaw{b}{tt}")
                nc.scalar.copy(pk[:, :SRC], x[:])
                negm = sm.tile([P, 1], F32, tag="negm")
                nc.vector.tensor_reduce(negm[:], x[:], axis=AXL.X,
                                        op=ALU.max, negate=True)
                ex = sm.tile([P, SRC], FP16, tag="ex")
                nc.scalar.activation(ex[:], x[:], ACT_EXP, bias=negm[:],
                                     scale=1.0)
                rowsum = sm.tile([P, 1], F32, tag="rowsum")
                nc.vector.scalar_tensor_tensor(ex[:], ex[:], 1.0,
                                               maskb_hf[:],
                                               op0=ALU.mult, op1=ALU.mult,
                                               accum_out=rowsum[:])
                z = sm.tile([P, 1], F32, tag="z")
                nc.vector.tensor_scalar_add(z[:], rowsum[:], 1e-6)
                rz = sm.tile([P, 1], F32, tag="rz")
                nc.vector.reciprocal(rz[:], z[:])
                nc.vector.tensor_scalar_mul(pk[:, SRC:], ex[:], rz[:])
                nc.sync.dma_start(out=aeaw_d[b, tt * P:(tt + 1) * P, :],
                                  in_=pk[:])
                pks.append(pk)

            def emit_attn_transpose(k):
                # transpose attn on the DMA xbar (SBUF->SBUF): frees the PE
                # of 8 transposes per tile.  Issued from the Scalar DGE one
                # tile late so its attn-ready wait never blocks the FIFO
                # (issuing on Sync serialized behind blocked aeaw entries
                # and starved mm3 by ~4.5us/tile).
                attnT = sm.tile([P, nS, P], FP16, tag="attnT", bufs=8,
                                name=f"attnT{k}")
                nc.scalar.dma_start(out=attnT[:], in_=pks[k][:, SRC:],
                                    transpose=True)
                attnTs[k] = attnT

            for k in range(len(tiles)):
                en_ps = emit_mm2(k)
                emit_softmax(k, en_ps)
                if k > 0:
                    emit_attn_transpose(k - 1)
            emit_attn_transpose(len(tiles) - 1)

            def emit_mm3(k):
                b, tt = tiles[k]
                attnT = attnTs.pop(k)
                last = (k == len(tiles) - 1)
                ctx_ps = psp.tile([P, TRGD], F32, tag="ps", name=f"ctx{k}")
                for st in range(nS):
                    for h in range(2):
                        hs = slice(h * 512, (h + 1) * 512)
                        chain(nc.tensor.matmul(ctx_ps[:, hs],
                                               attnT[:, st, :],
                                               val_sb[b][:, st, hs],
                                               start=(st == 0),
                                               stop=(st == nS - 1)))
                ctx_sb = sm.tile([P, TRGD], FP16, tag="ctx_sb")
                rows = slice(tt * P, (tt + 1) * P)
                if last:
                    # split the final drain across two engines + two DMAs to
                    # shorten the end-of-kernel tail
                    nc.vector.tensor_copy(ctx_sb[:, :512], ctx_ps[:, :512])
                    nc.scalar.copy(ctx_sb[:, 512:], ctx_ps[:, 512:])
                    nc.sync.dma_start(out=ctx_d[b, rows, 0:512],
                                      in_=ctx_sb[:, :512])
                    nc.sync.dma_start(out=ctx_d[b, rows, 512:],
                                      in_=ctx_sb[:, 512:])
                else:
                    nc.scalar.copy(ctx_sb[:], ctx_ps[:])
                    nc.sync.dma_start(out=ctx_d[b, rows, :], in_=ctx_sb[:])

            for k in range(len(tiles)):
                emit_mm3(k)

    nc.compile()
    return nc


def kernel(hidden, encoder_outputs, encoder_value, encoder_mask, W):
    global LAST_EXEC_NS, LAST_RESULTS
    from concourse.bass_utils import run_bass_kernel_spmd

    if "nc" not in _cache:
        _cache["nc"] = _build()
    nc = _cache["nc"]

    hidden = np.ascontiguousarray(hidden, dtype=np.float32)
    encoder_outputs = np.ascontiguousarray(encoder_outputs, dtype=np.float32)
    encoder_value = np.ascontiguousarray(encoder_value, dtype=np.float32)
    encoder_mask = np.ascontiguousarray(encoder_mask, dtype=np.float32)
    W = np.ascontiguousarray(W, dtype=np.float32)

    w_hf = W.astype(np.float16)
    in_maps = []
    for c in range(NCORES):
        sl = slice(c * BPC, (c + 1) * BPC)
        hid2 = hidden[sl]  # (2, TRG, TRGD)
        hidT01 = np.concatenate([hid2[0].T, hid2[1].T], axis=1)
        # partition-major marshaling: x_d[b, p, tile, free] so each batch
        # loads with 2 contiguous DMAs
        encT = encoder_outputs[sl].transpose(0, 2, 1)  # (BPC, ENCD, SRC)
        encT_pm = encT.reshape(BPC, nE, P, SRC).transpose(0, 2, 1, 3)
        val_pm = encoder_value[sl].reshape(BPC, nS, P, TRGD).transpose(
            0, 2, 1, 3)
        in_maps.append({
            "hidT": np.ascontiguousarray(hidT01.astype(np.float16)),
            "w": w_hf,
            "encT": np.ascontiguousarray(encT_pm.astype(np.float16)),
            "val": np.ascontiguousarray(val_pm.astype(np.float16)),
            "mask": encoder_mask[sl][:, None, :],
        })

    trace = bool(int(os.environ.get("KERNEL_TRACE", "0")))
    res = run_bass_kernel_spmd(nc, in_maps, core_ids=list(range(NCORES)),
                               trace=trace)
    LAST_EXEC_NS = res.exec_time_ns
    LAST_RESULTS = res

    aeaw = [res.results[c]["aeaw"] for c in range(NCORES)]
    context = np.concatenate([res.results[c]["ctx"] for c in range(NCORES)],
                             axis=0).astype(np.float32)
    attn_energies = np.concatenate(
        [a[:, :, :SRC] for a in aeaw], axis=0).astype(np.float32)
    attn_weights = np.concatenate(
        [a[:, :, SRC:] for a in aeaw], axis=0).astype(np.float32)
    return context, attn_weights, attn_energies


# revision 18
# speedup vs baseline: 1.0841x; 1.0085x over previous
"""Trainium2 Bass kernel for nn_Attention (general-score attention with
masked softmax), data-parallel over batch across 8 NeuronCores.

Math (per batch), matching the reference exactly for {0,1} float masks:
    raw[t,s]  = sum_e (hidden @ W)[t,e] * enc[s,e]       (associativity trick:
                (hidden @ W) @ enc^T  ==  hidden @ (enc @ W^T)^T, saves 25%
                FLOPs and avoids materializing proj)
    attn_energies = raw * mask            (mask in {0,1} so mask^2 == mask)
    e = exp(x - max_s x) * mask
    attn = e / (sum_s e + 1e-6)
    context = attn @ enc_value

v3, from HAM/trace analysis of v1 (156us) and v2 (128us):
  - All three gemms in fp16 with f32 PSUM accumulation (measured end-to-end
    rel err ~2.4e-3, same as all-f32r).  Warm PE issue rate is ~259ns per
    512-wide 16-bit matmul; the PE clock-gates to 1.2GHz after any ~3.4us
    idle window, so the whole kernel is one gap-free PE instruction chain.
  - mm1 fuses BOTH batches into one moving operand and consumes
    (w[dt], hidT01[dt]) DMA pairs as they land dt-outer; the last two dt
    rounds go et-wise with the PSUM->SBUF drain emitted right after each
    et's stop so the next pass never waits on a drain (v2 lost ~4us there).
  - 8 junk transposes of the identity warm the HAM clock gate during the
    ~9us DMA/preamble dead time, so mm1 runs at 2.4GHz almost from the start.
  - attnT copies go on GpSimd (idle engine) - in v2 they queued behind the
    Scalar softmax backlog at the B->C boundary (~1.7us stall).
  - ae+aw are packed into one [128,2048] tile and one DMA per tile; encT
    and val are marshaled partition-major on the host so each loads with 2
    DMAs per batch.  The Sync engine issues each dma_start serially at
    ~0.73us, so fewer+bigger transfers keep it off the critical path.
  - the final tile's ctx drain is split across DVE+Scalar with two DMA
    halves to shorten the end-of-kernel tail.
"""
import os

import numpy as np

B, TRG, SRC, ENCD, TRGD = 16, 512, 1024, 1024, 1024
NCORES = 8
BPC = B // NCORES  # batches per core
P = 128
nD = TRGD // P   # 8 contraction tiles over d
nE = ENCD // P   # 8 over e
nS = SRC // P    # 8 over s
nT = TRG // P    # 4 t-tiles per batch
TRG2 = BPC * TRG  # both batches fused along t: 1024

_cache = {}

LAST_EXEC_NS = None
LAST_RESULTS = None


def _build():
    import bass_rust
    import concourse.mybir as mybir
    import concourse.tile as tile
    from concourse import bacc
    from concourse.masks import make_identity

    _add_dep = bass_rust.add_dep_helper

    F32 = mybir.dt.float32
    FP16 = mybir.dt.float16
    ALU = mybir.AluOpType
    AXL = mybir.AxisListType
    ACT_EXP = mybir.ActivationFunctionType.Exp

    nc = bacc.Bacc("TRN2", target_bir_lowering=False, debug=False)

    hidT_d = nc.dram_tensor("hidT", (TRGD, TRG2), FP16, kind="ExternalInput")
    w_d = nc.dram_tensor("w", (TRGD, ENCD), FP16, kind="ExternalInput")
    encT_d = nc.dram_tensor("encT", (BPC, P, nE, SRC), FP16,
                            kind="ExternalInput")
    val_d = nc.dram_tensor("val", (BPC, P, nS, TRGD), FP16,
                           kind="ExternalInput")
    mask_d = nc.dram_tensor("mask", (BPC, 1, SRC), F32, kind="ExternalInput")
    aeaw_d = nc.dram_tensor("aeaw", (BPC, TRG, 2 * SRC), FP16,
                            kind="ExternalOutput")
    ctx_d = nc.dram_tensor("ctx", (BPC, TRG, TRGD), FP16,
                           kind="ExternalOutput")

    with tile.TileContext(nc) as tc:
        with (
            tc.tile_pool(name="const", bufs=1) as const,
            tc.tile_pool(name="wp", bufs=1) as wp,
            tc.tile_pool(name="big", bufs=1) as big,
            tc.tile_pool(name="sm", bufs=2) as sm,
            tc.tile_pool(name="ps", bufs=4, space="PSUM") as psp,
        ):
            ident = const.tile([P, P], F32)
            make_identity(nc, ident[:])
            identh = const.tile([P, P], FP16)
            nc.vector.tensor_copy(identh[:], ident[:])

            # PE program order is pinned with an explicit linear chain so the
            # scheduler can never interleave accumulation groups or delay a
            # group's stop.
            pe_prev = [None]

            def chain(mm):
                if pe_prev[0] is not None:
                    _add_dep(mm.ins, pe_prev[0].ins, sync=False,
                             reason="pe order")
                pe_prev[0] = mm
                return mm

            # ---- loads (issue order == consumption order) ----
            w_sb = [wp.tile([P, ENCD], FP16, tag=f"w{i}", name=f"w_sb{i}")
                    for i in range(nD)]
            hidT_sb = [big.tile([P, TRG2], FP16, tag=f"hidT{i}",
                                name=f"hidT_sb{i}") for i in range(nD)]
            for i in range(nD):
                nc.sync.dma_start(out=w_sb[i][:], in_=w_d[i * P:(i + 1) * P, :])
                nc.sync.dma_start(out=hidT_sb[i][:],
                                  in_=hidT_d[i * P:(i + 1) * P, :])
            maskbs = []
            for b in range(BPC):
                maskb = sm.tile([P, SRC], F32, tag="maskb", name=f"maskb{b}")
                nc.sync.dma_start(out=maskb[:],
                                  in_=mask_d[b].to_broadcast((P, SRC)))
                maskb_hf = sm.tile([P, SRC], FP16, tag="maskb_hf",
                                   name=f"maskb_hf{b}")
                nc.vector.tensor_copy(maskb_hf[:], maskb[:])
                maskbs.append((maskb, maskb_hf))
            encT_sb = []
            val_sb = []
            for b in range(BPC):
                e_t = big.tile([P, nE, SRC], FP16, tag="encT", bufs=2,
                               name=f"encT_sb{b}")
                for g in range(2):
                    gs = slice(g * (nE // 2), (g + 1) * (nE // 2))
                    nc.sync.dma_start(out=e_t[:, gs, :],
                                      in_=encT_d[b, :, gs, :])
                v_t = big.tile([P, nS, TRGD], FP16, tag="val", bufs=2,
                               name=f"val_sb{b}")
                for g in range(2):
                    gs = slice(g * (nS // 2), (g + 1) * (nS // 2))
                    nc.sync.dma_start(out=v_t[:, gs, :],
                                      in_=val_d[b, :, gs, :])
                encT_sb.append(e_t)
                val_sb.append(v_t)

            # ---- mm1: HpT[e, t01] = sum_d W[d,e] * hidT01[d, t01] ----
            # two half-passes of 4 et each (4 psum bufs per pass).  dt-outer
            # for DMA pair-wise consumption, but the last two dt rounds go
            # et-wise with the drain right after each stop so ring slots free
            # up staggered instead of all at the end.
            HpT = big.tile([P, nE, TRG2], FP16, tag="HpT", name="HpT")
            drain_eng = [0]

            def drain(dst, src):
                if drain_eng[0] % 2 == 0:
                    nc.vector.tensor_copy(dst, src)
                else:
                    nc.scalar.copy(dst, src)
                drain_eng[0] += 1

            def mm1_mm(pp, dt, et):
                for h in range(2):
                    hs = slice(h * 512, (h + 1) * 512)
                    chain(nc.tensor.matmul(
                        pp[:, hs], w_sb[dt][:, et * P:(et + 1) * P],
                        hidT_sb[dt][:, hs],
                        start=(dt == 0), stop=(dt == nD - 1)))

            def emit_mm1_pass(ets, warm=False):
                pps = [psp.tile([P, TRG2], F32, tag="ps", name=f"mm1ps{et}")
                       for et in ets]
                if warm:
                    # junk transposes of the identity: keep the PE busy
                    # during the DMA/preamble dead time so the HAM clock
                    # gate is warm (2.4GHz) when real work arrives.  The
                    # garbage psum is overwritten by mm1's start=True.
                    junk_view = pps[0][:].bitcast(FP16)
                    for _ in range(24):
                        chain(nc.tensor.matmul(
                            junk_view[:, 0:P], identh[:], identh[:],
                            is_transpose=True, skip_group_check=True))
                for dt in range(nD - 2):
                    for i, et in enumerate(ets):
                        mm1_mm(pps[i], dt, et)
                for i, et in enumerate(ets):
                    for dt in (nD - 2, nD - 1):
                        mm1_mm(pps[i], dt, et)
                    drain(HpT[:, et, :], pps[i][:])

            emit_mm1_pass(range(0, nE // 2), warm=True)
            emit_mm1_pass(range(nE // 2, nE))

            # ---- mm2 + masked softmax over 8 supertiles (b, tt) ----
            tiles = [(b, tt) for b in range(BPC) for tt in range(nT)]
            pks = []
            attnTs = {}

            def emit_mm2(k):
                b, tt = tiles[k]
                ts = slice(b * TRG + tt * P, b * TRG + (tt + 1) * P)
                en_ps = psp.tile([P, SRC], F32, tag="ps", name=f"en{b}{tt}")
                for et in range(nE):
                    for h in range(2):
                        hs = slice(h * 512, (h + 1) * 512)
                        chain(nc.tensor.matmul(en_ps[:, hs], HpT[:, et, ts],
                                               encT_sb[b][:, et, hs],
                                               start=(et == 0),
                                               stop=(et == nE - 1)))
                return en_ps

            def emit_softmax(k, en_ps):
                b, tt = tiles[k]
                maskb, maskb_hf = maskbs[b]
                x = sm.tile([P, SRC], F32, tag="x")
                nc.vector.tensor_mul(x[:], en_ps[:], maskb[:])
                # packed [ae | attn] tile: one output DMA per supertile
                pk = sm.tile([P, 2 * SRC], FP16, tag="aeaw", bufs=7,
                             name=f"aeaw{b}{tt}")
                nc.scalar.copy(pk[:, :SRC], x[:])
                negm = sm.tile([P, 1], F32, tag="negm")
                nc.vector.tensor_reduce(negm[:], x[:], axis=AXL.X,
                                        op=ALU.max, negate=True)
                ex = sm.tile([P, SRC], FP16, tag="ex")
                nc.scalar.activation(ex[:], x[:], ACT_EXP, bias=negm[:],
                                     scale=1.0)
                rowsum = sm.tile([P, 1], F32, tag="rowsum")
                nc.vector.scalar_tensor_tensor(ex[:], ex[:], 1.0,
                                               maskb_hf[:],
                                               op0=ALU.mult, op1=ALU.mult,
                                               accum_out=rowsum[:])
                z = sm.tile([P, 1], F32, tag="z")
                nc.vector.tensor_scalar_add(z[:], rowsum[:], 1e-6)
                rz = sm.tile([P, 1], F32, tag="rz")
                nc.vector.reciprocal(rz[:], z[:])
                nc.vector.tensor_scalar_mul(pk[:, SRC:], ex[:], rz[:])
                nc.sync.dma_start(out=aeaw_d[b, tt * P:(tt + 1) * P, :],
                                  in_=pk[:])
                pks.append(pk)

            def emit_attn_transpose(k):
                # transpose attn on the DMA xbar (SBUF->SBUF): frees the PE
                # of 8 transposes per tile.  Issued from the Scalar DGE one
                # tile late so its attn-ready wait never blocks the FIFO
                # (issuing on Sync serialized behind blocked aeaw entries
                # and starved mm3 by ~4.5us/tile).
                attnT = sm.tile([P, nS, P], FP16, tag="attnT", bufs=8,
                                name=f"attnT{k}")
                nc.scalar.dma_start(out=attnT[:], in_=pks[k][:, SRC:],
                                    transpose=True)
                attnTs[k] = attnT

            for k in range(len(tiles)):
                en_ps = emit_mm2(k)
                emit_softmax(k, en_ps)
                if k > 0:
                    emit_attn_transpose(k - 1)
            emit_attn_transpose(len(tiles) - 1)

            def emit_mm3(k):
                b, tt = tiles[k]
                attnT = attnTs.pop(k)
                last = (k == len(tiles) - 1)
                ctx_ps = psp.tile([P, TRGD], F32, tag="ps", name=f"ctx{k}")
                for st in range(nS):
                    for h in range(2):
                        hs = slice(h * 512, (h + 1) * 512)
                        chain(nc.tensor.matmul(ctx_ps[:, hs],
                                               attnT[:, st, :],
                                               val_sb[b][:, st, hs],
                                               start=(st == 0),
                                               stop=(st == nS - 1)))
                ctx_sb = sm.tile([P, TRGD], FP16, tag="ctx_sb")
                rows = slice(tt * P, (tt + 1) * P)
                if last:
                    # split the final drain across two engines + two DMAs to
                    # shorten the end-of-kernel tail
                    nc.vector.tensor_copy(ctx_sb[:, :512], ctx_ps[:, :512])
                    nc.scalar.copy(ctx_sb[:, 512:], ctx_ps[:, 512:])
                    nc.sync.dma_start(out=ctx_d[b, rows, 0:512],
                                      in_=ctx_sb[:, :512])
                    nc.sync.dma_start(out=ctx_d[b, rows, 512:],
                                      in_=ctx_sb[:, 512:])
                else:
                    # DVE, not Scalar: the Scalar FIFO still holds late
                    # attn-transpose issues during early mm3 tiles
                    nc.vector.tensor_copy(ctx_sb[:], ctx_ps[:])
                    nc.sync.dma_start(out=ctx_d[b, rows, :], in_=ctx_sb[:])

            for k in range(len(tiles)):
                emit_mm3(k)

    nc.compile()
    return nc


def kernel(hidden, encoder_outputs, encoder_value, encoder_mask, W):
    global LAST_EXEC_NS, LAST_RESULTS
    from concourse.bass_utils import run_bass_kernel_spmd

    if "nc" not in _cache:
        _cache["nc"] = _build()
    nc = _cache["nc"]

    hidden = np.ascontiguousarray(hidden, dtype=np.float32)
    encoder_outputs = np.ascontiguousarray(encoder_outputs, dtype=np.float32)
    encoder_value = np.ascontiguousarray(encoder_value, dtype=np.float32)
    encoder_mask = np.ascontiguousarray(encoder_mask, dtype=np.float32)
    W = np.ascontiguousarray(W, dtype=np.float32)

    w_hf = W.astype(np.float16)
    in_maps = []
    for c in range(NCORES):
        sl = slice(c * BPC, (c + 1) * BPC)
        hid2 = hidden[sl]  # (2, TRG, TRGD)
        hidT01 = np.concatenate([hid2[0].T, hid2[1].T], axis=1)
        # partition-major marshaling: x_d[b, p, tile, free] so each batch
        # loads with 2 contiguous DMAs
        encT = encoder_outputs[sl].transpose(0, 2, 1)  # (BPC, ENCD, SRC)
        encT_pm = encT.reshape(BPC, nE, P, SRC).transpose(0, 2, 1, 3)
        val_pm = encoder_value[sl].reshape(BPC, nS, P, TRGD).transpose(
            0, 2, 1, 3)
        in_maps.append({
            "hidT": np.ascontiguousarray(hidT01.astype(np.float16)),
            "w": w_hf,
            "encT": np.ascontiguousarray(encT_pm.astype(np.float16)),
            "val": np.ascontiguousarray(val_pm.astype(np.float16)),
            "mask": encoder_mask[sl][:, None, :],
        })

    trace = bool(int(os.environ.get("KERNEL_TRACE", "0")))
    res = run_bass_kernel_spmd(nc, in_maps, core_ids=list(range(NCORES)),
                               trace=trace)
    LAST_EXEC_NS = res.exec_time_ns
    LAST_RESULTS = res

    aeaw = [res.results[c]["aeaw"] for c in range(NCORES)]
    context = np.concatenate([res.results[c]["ctx"] for c in range(NCORES)],
                             axis=0).astype(np.float32)
    attn_energies = np.concatenate(
        [a[:, :, :SRC] for a in aeaw], axis=0).astype(np.float32)
    attn_weights = np.concatenate(
        [a[:, :, SRC:] for a in aeaw], axis=0).astype(np.float32)
    return context, attn_weights, attn_energies


# revision 20
# speedup vs baseline: 1.1811x; 1.0895x over previous
"""Trainium2 Bass kernel for nn_Attention (general-score attention with
masked softmax), data-parallel over batch across 8 NeuronCores.

Math (per batch), matching the reference exactly for {0,1} float masks:
    raw[t,s]  = sum_e (hidden @ W)[t,e] * enc[s,e]       (associativity trick:
                (hidden @ W) @ enc^T  ==  hidden @ (enc @ W^T)^T, saves 25%
                FLOPs and avoids materializing proj)
    attn_energies = raw * mask            (mask in {0,1} so mask^2 == mask)
    e = exp(x - max_s x) * mask
    attn = e / (sum_s e + 1e-6)
    context = attn @ enc_value

v3, from HAM/trace analysis of v1 (156us) and v2 (128us):
  - All three gemms in fp16 with f32 PSUM accumulation (measured end-to-end
    rel err ~2.4e-3, same as all-f32r).  Warm PE issue rate is ~259ns per
    512-wide 16-bit matmul; the PE clock-gates to 1.2GHz after any ~3.4us
    idle window, so the whole kernel is one gap-free PE instruction chain.
  - mm1 fuses BOTH batches into one moving operand and consumes
    (w[dt], hidT01[dt]) DMA pairs as they land dt-outer; the last two dt
    rounds go et-wise with the PSUM->SBUF drain emitted right after each
    et's stop so the next pass never waits on a drain (v2 lost ~4us there).
  - 8 junk transposes of the identity warm the HAM clock gate during the
    ~9us DMA/preamble dead time, so mm1 runs at 2.4GHz almost from the start.
  - attnT copies go on GpSimd (idle engine) - in v2 they queued behind the
    Scalar softmax backlog at the B->C boundary (~1.7us stall).
  - ae+aw are packed into one [128,2048] tile and one DMA per tile; encT
    and val are marshaled partition-major on the host so each loads with 2
    DMAs per batch.  The Sync engine issues each dma_start serially at
    ~0.73us, so fewer+bigger transfers keep it off the critical path.
  - the final tile's ctx drain is split across DVE+Scalar with two DMA
    halves to shorten the end-of-kernel tail.
"""
import os

import numpy as np

B, TRG, SRC, ENCD, TRGD = 16, 512, 1024, 1024, 1024
NCORES = 8
BPC = B // NCORES  # batches per core
P = 128
nD = TRGD // P   # 8 contraction tiles over d
nE = ENCD // P   # 8 over e
nS = SRC // P    # 8 over s
nT = TRG // P    # 4 t-tiles per batch
TRG2 = BPC * TRG  # both batches fused along t: 1024

_cache = {}

LAST_EXEC_NS = None
LAST_RESULTS = None


def _build():
    import bass_rust
    import concourse.mybir as mybir
    import concourse.tile as tile
    from concourse import bacc
    from concourse.masks import make_identity

    _add_dep = bass_rust.add_dep_helper

    F32 = mybir.dt.float32
    FP16 = mybir.dt.float16
    ALU = mybir.AluOpType
    AXL = mybir.AxisListType
    ACT_EXP = mybir.ActivationFunctionType.Exp

    nc = bacc.Bacc("TRN2", target_bir_lowering=False, debug=False)

    hidT_d = nc.dram_tensor("hidT", (TRGD, TRG2), FP16, kind="ExternalInput")
    w_d = nc.dram_tensor("w", (TRGD, ENCD), FP16, kind="ExternalInput")
    encT_d = nc.dram_tensor("encT", (BPC, P, nE, SRC), FP16,
                            kind="ExternalInput")
    val_d = nc.dram_tensor("val", (BPC, P, nS, TRGD), FP16,
                           kind="ExternalInput")
    mask_d = nc.dram_tensor("mask", (BPC, 1, SRC), F32, kind="ExternalInput")
    aeaw_d = nc.dram_tensor("aeaw", (BPC, TRG, 2 * SRC), FP16,
                            kind="ExternalOutput")
    ctx_d = nc.dram_tensor("ctx", (BPC, TRG, TRGD), FP16,
                           kind="ExternalOutput")

    with tile.TileContext(nc) as tc:
        with (
            tc.tile_pool(name="const", bufs=1) as const,
            tc.tile_pool(name="wp", bufs=1) as wp,
            tc.tile_pool(name="big", bufs=1) as big,
            tc.tile_pool(name="sm", bufs=2) as sm,
            tc.tile_pool(name="ps", bufs=4, space="PSUM") as psp,
        ):
            ident = const.tile([P, P], F32)
            make_identity(nc, ident[:])
            identh = const.tile([P, P], FP16)
            nc.vector.tensor_copy(identh[:], ident[:])

            # PE program order is pinned with an explicit linear chain so the
            # scheduler can never interleave accumulation groups or delay a
            # group's stop.
            pe_prev = [None]

            def chain(mm):
                if pe_prev[0] is not None:
                    _add_dep(mm.ins, pe_prev[0].ins, sync=False,
                             reason="pe order")
                pe_prev[0] = mm
                return mm

            # ---- loads (issue order == consumption order) ----
            w_sb = [wp.tile([P, ENCD], FP16, tag=f"w{i}", name=f"w_sb{i}")
                    for i in range(nD)]
            hidT_sb = [big.tile([P, TRG2], FP16, tag=f"hidT{i}",
                                name=f"hidT_sb{i}") for i in range(nD)]
            for i in range(nD):
                nc.sync.dma_start(out=w_sb[i][:], in_=w_d[i * P:(i + 1) * P, :])
                nc.sync.dma_start(out=hidT_sb[i][:],
                                  in_=hidT_d[i * P:(i + 1) * P, :])
            maskbs = []
            for b in range(BPC):
                maskb = sm.tile([P, SRC], F32, tag="maskb", name=f"maskb{b}")
                nc.sync.dma_start(out=maskb[:],
                                  in_=mask_d[b].to_broadcast((P, SRC)))
                maskb_hf = sm.tile([P, SRC], FP16, tag="maskb_hf",
                                   name=f"maskb_hf{b}")
                nc.vector.tensor_copy(maskb_hf[:], maskb[:])
                maskbs.append((maskb, maskb_hf))
            encT_sb = []
            val_sb = []
            for b in range(BPC):
                e_t = big.tile([P, nE, SRC], FP16, tag="encT", bufs=2,
                               name=f"encT_sb{b}")
                for g in range(2):
                    gs = slice(g * (nE // 2), (g + 1) * (nE // 2))
                    nc.sync.dma_start(out=e_t[:, gs, :],
                                      in_=encT_d[b, :, gs, :])
                v_t = big.tile([P, nS, TRGD], FP16, tag="val", bufs=2,
                               name=f"val_sb{b}")
                for g in range(2):
                    gs = slice(g * (nS // 2), (g + 1) * (nS // 2))
                    nc.sync.dma_start(out=v_t[:, gs, :],
                                      in_=val_d[b, :, gs, :])
                encT_sb.append(e_t)
                val_sb.append(v_t)

            # ---- mm1: HpT[e, t01] = sum_d W[d,e] * hidT01[d, t01] ----
            # two half-passes of 4 et each (4 psum bufs per pass).  dt-outer
            # for DMA pair-wise consumption, but the last two dt rounds go
            # et-wise with the drain right after each stop so ring slots free
            # up staggered instead of all at the end.
            HpT = big.tile([P, nE, TRG2], FP16, tag="HpT", name="HpT")
            drain_eng = [0]

            def drain(dst, src):
                if drain_eng[0] % 2 == 0:
                    nc.vector.tensor_copy(dst, src)
                else:
                    nc.scalar.copy(dst, src)
                drain_eng[0] += 1

            def mm1_mm(pp, dt, et):
                for h in range(2):
                    hs = slice(h * 512, (h + 1) * 512)
                    chain(nc.tensor.matmul(
                        pp[:, hs], w_sb[dt][:, et * P:(et + 1) * P],
                        hidT_sb[dt][:, hs],
                        start=(dt == 0), stop=(dt == nD - 1)))

            def emit_mm1_pass(ets, warm=False):
                pps = [psp.tile([P, TRG2], F32, tag="ps", name=f"mm1ps{et}")
                       for et in ets]
                if warm:
                    # junk transposes of the identity: keep the PE busy
                    # during the DMA/preamble dead time so the HAM clock
                    # gate is warm (2.4GHz) when real work arrives.  The
                    # garbage psum is overwritten by mm1's start=True.
                    junk_view = pps[0][:].bitcast(FP16)
                    for _ in range(24):
                        chain(nc.tensor.matmul(
                            junk_view[:, 0:P], identh[:], identh[:],
                            is_transpose=True, skip_group_check=True))
                for dt in range(nD - 2):
                    for i, et in enumerate(ets):
                        mm1_mm(pps[i], dt, et)
                for i, et in enumerate(ets):
                    for dt in (nD - 2, nD - 1):
                        mm1_mm(pps[i], dt, et)
                    drain(HpT[:, et, :], pps[i][:])

            emit_mm1_pass(range(0, nE // 2), warm=True)
            emit_mm1_pass(range(nE // 2, nE))

            # ---- mm2 + masked softmax over 8 supertiles (b, tt) ----
            tiles = [(b, tt) for b in range(BPC) for tt in range(nT)]
            pks = []
            attnTs = {}

            def emit_mm2(k):
                b, tt = tiles[k]
                ts = slice(b * TRG + tt * P, b * TRG + (tt + 1) * P)
                en_ps = psp.tile([P, SRC], F32, tag="ps", name=f"en{b}{tt}")
                for et in range(nE):
                    for h in range(2):
                        hs = slice(h * 512, (h + 1) * 512)
                        chain(nc.tensor.matmul(en_ps[:, hs], HpT[:, et, ts],
                                               encT_sb[b][:, et, hs],
                                               start=(et == 0),
                                               stop=(et == nE - 1)))
                return en_ps

            def emit_softmax(k, en_ps):
                b, tt = tiles[k]
                maskb, maskb_hf = maskbs[b]
                x = sm.tile([P, SRC], F32, tag="x")
                nc.vector.tensor_mul(x[:], en_ps[:], maskb[:])
                # packed [ae | attn] tile: one output DMA per supertile
                pk = sm.tile([P, 2 * SRC], FP16, tag="aeaw", bufs=7,
                             name=f"aeaw{b}{tt}")
                nc.scalar.copy(pk[:, :SRC], x[:])
                negm = sm.tile([P, 1], F32, tag="negm")
                nc.vector.tensor_reduce(negm[:], x[:], axis=AXL.X,
                                        op=ALU.max, negate=True)
                ex = sm.tile([P, SRC], FP16, tag="ex")
                nc.scalar.activation(ex[:], x[:], ACT_EXP, bias=negm[:],
                                     scale=1.0)
                rowsum = sm.tile([P, 1], F32, tag="rowsum")
                nc.vector.scalar_tensor_tensor(ex[:], ex[:], 1.0,
                                               maskb_hf[:],
                                               op0=ALU.mult, op1=ALU.mult,
                                               accum_out=rowsum[:])
                z = sm.tile([P, 1], F32, tag="z")
                nc.vector.tensor_scalar_add(z[:], rowsum[:], 1e-6)
                rz = sm.tile([P, 1], F32, tag="rz")
                nc.vector.reciprocal(rz[:], z[:])
                nc.vector.tensor_scalar_mul(pk[:, SRC:], ex[:], rz[:])
                nc.sync.dma_start(out=aeaw_d[b, tt * P:(tt + 1) * P, :],
                                  in_=pk[:])
                pks.append(pk)

            def emit_tr(k):
                # PE transposes: the DMA-xbar alternative measures ~5-6us
                # per [128,1024] tile on hardware and serializes — PE does
                # all 8 in ~0.9us
                attn = pks[k][:, SRC:]
                trp = psp.tile([P, SRC], F32, tag="ps", name=f"tr{k}")
                trh = trp[:].bitcast(FP16)
                for st in range(nS):
                    chain(nc.tensor.transpose(trh[:, st * P:(st + 1) * P],
                                              attn[:, st * P:(st + 1) * P],
                                              identh[:]))
                attnT = sm.tile([P, nS, P], FP16, tag="attnT",
                                name=f"attnT{k}")
                nc.scalar.copy(attnT[:], trh[:, :SRC])
                attnTs[k] = attnT

            for k in range(len(tiles)):
                en_ps = emit_mm2(k)
                if k == len(tiles) - 1:
                    # emit tr(T0) BEFORE the last softmax's engine ops: its
                    # attnT copy then sits ahead of them in the Scalar FIFO
                    # instead of queuing behind
                    emit_tr(0)
                emit_softmax(k, en_ps)

            def emit_mm3(k):
                b, tt = tiles[k]
                attnT = attnTs.pop(k)
                last = (k == len(tiles) - 1)
                ctx_ps = psp.tile([P, TRGD], F32, tag="ps", name=f"ctx{k}")
                for st in range(nS):
                    for h in range(2):
                        hs = slice(h * 512, (h + 1) * 512)
                        chain(nc.tensor.matmul(ctx_ps[:, hs],
                                               attnT[:, st, :],
                                               val_sb[b][:, st, hs],
                                               start=(st == 0),
                                               stop=(st == nS - 1)))
                ctx_sb = sm.tile([P, TRGD], FP16, tag="ctx_sb")
                rows = slice(tt * P, (tt + 1) * P)
                if last:
                    # split the final drain across two engines + two DMAs to
                    # shorten the end-of-kernel tail
                    nc.vector.tensor_copy(ctx_sb[:, :512], ctx_ps[:, :512])
                    nc.scalar.copy(ctx_sb[:, 512:], ctx_ps[:, 512:])
                    nc.sync.dma_start(out=ctx_d[b, rows, 0:512],
                                      in_=ctx_sb[:, :512])
                    nc.sync.dma_start(out=ctx_d[b, rows, 512:],
                                      in_=ctx_sb[:, 512:])
                else:
                    # DVE, not Scalar: the Scalar FIFO still holds late
                    # attn-transpose issues during early mm3 tiles
                    nc.vector.tensor_copy(ctx_sb[:], ctx_ps[:])
                    nc.sync.dma_start(out=ctx_d[b, rows, :], in_=ctx_sb[:])

            for k in range(len(tiles)):
                if k + 1 < len(tiles):
                    emit_tr(k + 1)
                emit_mm3(k)

    nc.compile()
    return nc


def kernel(hidden, encoder_outputs, encoder_value, encoder_mask, W):
    global LAST_EXEC_NS, LAST_RESULTS
    from concourse.bass_utils import run_bass_kernel_spmd

    if "nc" not in _cache:
        _cache["nc"] = _build()
    nc = _cache["nc"]

    hidden = np.ascontiguousarray(hidden, dtype=np.float32)
    encoder_outputs = np.ascontiguousarray(encoder_outputs, dtype=np.float32)
    encoder_value = np.ascontiguousarray(encoder_value, dtype=np.float32)
    encoder_mask = np.ascontiguousarray(encoder_mask, dtype=np.float32)
    W = np.ascontiguousarray(W, dtype=np.float32)

    w_hf = W.astype(np.float16)
    in_maps = []
    for c in range(NCORES):
        sl = slice(c * BPC, (c + 1) * BPC)
        hid2 = hidden[sl]  # (2, TRG, TRGD)
        hidT01 = np.concatenate([hid2[0].T, hid2[1].T], axis=1)
        # partition-major marshaling: x_d[b, p, tile, free] so each batch
        # loads with 2 contiguous DMAs
        encT = encoder_outputs[sl].transpose(0, 2, 1)  # (BPC, ENCD, SRC)
        encT_pm = encT.reshape(BPC, nE, P, SRC).transpose(0, 2, 1, 3)
        val_pm = encoder_value[sl].reshape(BPC, nS, P, TRGD).transpose(
            0, 2, 1, 3)
        in_maps.append({
            "hidT": np.ascontiguousarray(hidT01.astype(np.float16)),
            "w": w_hf,
            "encT": np.ascontiguousarray(encT_pm.astype(np.float16)),
            "val": np.ascontiguousarray(val_pm.astype(np.float16)),
            "mask": encoder_mask[sl][:, None, :],
        })

    trace = bool(int(os.environ.get("KERNEL_TRACE", "0")))
    res = run_bass_kernel_spmd(nc, in_maps, core_ids=list(range(NCORES)),
                               trace=trace)
    LAST_EXEC_NS = res.exec_time_ns
    LAST_RESULTS = res

    aeaw = [res.results[c]["aeaw"] for c in range(NCORES)]
    context = np.concatenate([res.results[c]["ctx"] for c in range(NCORES)],
                             axis=0).astype(np.float32)
    attn_energies = np.concatenate(
        [a[:, :, :SRC] for a in aeaw], axis=0).astype(np.float32)
    attn_weights = np.concatenate(
        [a[:, :, SRC:] for a in aeaw], axis=0).astype(np.float32)
    return context, attn_weights, attn_energies


# revision 23
# speedup vs baseline: 1.2097x; 1.0242x over previous
"""Trainium2 Bass kernel for nn_Attention (general-score attention with
masked softmax), data-parallel over batch across 8 NeuronCores.

Math (per batch), matching the reference exactly for {0,1} float masks:
    raw[t,s]  = sum_e (hidden @ W)[t,e] * enc[s,e]       (associativity trick:
                (hidden @ W) @ enc^T  ==  hidden @ (enc @ W^T)^T, saves 25%
                FLOPs and avoids materializing proj)
    attn_energies = raw * mask            (mask in {0,1} so mask^2 == mask)
    e = exp(x - max_s x) * mask
    attn = e / (sum_s e + 1e-6)
    context = attn @ enc_value

v3, from HAM/trace analysis of v1 (156us) and v2 (128us):
  - All three gemms in fp16 with f32 PSUM accumulation (measured end-to-end
    rel err ~2.4e-3, same as all-f32r).  Warm PE issue rate is ~259ns per
    512-wide 16-bit matmul; the PE clock-gates to 1.2GHz after any ~3.4us
    idle window, so the whole kernel is one gap-free PE instruction chain.
  - mm1 fuses BOTH batches into one moving operand and consumes
    (w[dt], hidT01[dt]) DMA pairs as they land dt-outer; the last two dt
    rounds go et-wise with the PSUM->SBUF drain emitted right after each
    et's stop so the next pass never waits on a drain (v2 lost ~4us there).
  - 8 junk transposes of the identity warm the HAM clock gate during the
    ~9us DMA/preamble dead time, so mm1 runs at 2.4GHz almost from the start.
  - attnT copies go on GpSimd (idle engine) - in v2 they queued behind the
    Scalar softmax backlog at the B->C boundary (~1.7us stall).
  - ae+aw are packed into one [128,2048] tile and one DMA per tile; encT
    and val are marshaled partition-major on the host so each loads with 2
    DMAs per batch.  The Sync engine issues each dma_start serially at
    ~0.73us, so fewer+bigger transfers keep it off the critical path.
  - the final tile's ctx drain is split across DVE+Scalar with two DMA
    halves to shorten the end-of-kernel tail.
"""
import os

import numpy as np

B, TRG, SRC, ENCD, TRGD = 16, 512, 1024, 1024, 1024
NCORES = 8
BPC = B // NCORES  # batches per core
P = 128
nD = TRGD // P   # 8 contraction tiles over d
nE = ENCD // P   # 8 over e
nS = SRC // P    # 8 over s
nT = TRG // P    # 4 t-tiles per batch
TRG2 = BPC * TRG  # both batches fused along t: 1024

_cache = {}

LAST_EXEC_NS = None
LAST_RESULTS = None


def _build():
    import bass_rust
    import concourse.mybir as mybir
    import concourse.tile as tile
    from concourse import bacc
    from concourse.masks import make_identity

    _add_dep = bass_rust.add_dep_helper

    F32 = mybir.dt.float32
    FP16 = mybir.dt.float16
    ALU = mybir.AluOpType
    AXL = mybir.AxisListType
    ACT_EXP = mybir.ActivationFunctionType.Exp

    nc = bacc.Bacc("TRN2", target_bir_lowering=False, debug=False)

    hidT_d = nc.dram_tensor("hidT", (TRGD, TRG2), FP16, kind="ExternalInput")
    w_d = nc.dram_tensor("w", (TRGD, ENCD), FP16, kind="ExternalInput")
    encT_d = nc.dram_tensor("encT", (BPC, P, nE, SRC), FP16,
                            kind="ExternalInput")
    val_d = nc.dram_tensor("val", (BPC, P, nS, TRGD), FP16,
                           kind="ExternalInput")
    mask_d = nc.dram_tensor("mask", (BPC, 1, SRC), F32, kind="ExternalInput")
    aeaw_d = nc.dram_tensor("aeaw", (BPC, TRG, 2 * SRC), FP16,
                            kind="ExternalOutput")
    ctx_d = nc.dram_tensor("ctx", (BPC, TRG, TRGD), FP16,
                           kind="ExternalOutput")

    with tile.TileContext(nc) as tc:
        with (
            tc.tile_pool(name="const", bufs=1) as const,
            tc.tile_pool(name="wp", bufs=1) as wp,
            tc.tile_pool(name="big", bufs=1) as big,
            tc.tile_pool(name="sm", bufs=2) as sm,
            tc.tile_pool(name="ps", bufs=4, space="PSUM") as psp,
        ):
            ident = const.tile([P, P], F32)
            make_identity(nc, ident[:])
            identh = const.tile([P, P], FP16)
            nc.vector.tensor_copy(identh[:], ident[:])

            # PE program order is pinned with an explicit linear chain so the
            # scheduler can never interleave accumulation groups or delay a
            # group's stop.
            pe_prev = [None]

            def chain(mm):
                if pe_prev[0] is not None:
                    _add_dep(mm.ins, pe_prev[0].ins, sync=False,
                             reason="pe order")
                pe_prev[0] = mm
                return mm

            # ---- loads (issue order == consumption order) ----
            w_sb = [wp.tile([P, ENCD], FP16, tag=f"w{i}", name=f"w_sb{i}")
                    for i in range(nD)]
            hidT_sb = [big.tile([P, TRG2], FP16, tag=f"hidT{i}",
                                name=f"hidT_sb{i}") for i in range(nD)]
            # first two w tiles split in half: pass1 only reads w cols 0:512,
            # so the first (w, hidT) pairs land ~1us sooner and the PE ramp
            # starts earlier; the second halves are issued after the pairs
            for i in range(nD):
                rows = slice(i * P, (i + 1) * P)
                if i < 2:
                    nc.sync.dma_start(out=w_sb[i][:, 0:512],
                                      in_=w_d[rows, 0:512])
                else:
                    nc.sync.dma_start(out=w_sb[i][:], in_=w_d[rows, :])
                nc.sync.dma_start(out=hidT_sb[i][:], in_=hidT_d[rows, :])
            for i in range(2):
                rows = slice(i * P, (i + 1) * P)
                nc.sync.dma_start(out=w_sb[i][:, 512:], in_=w_d[rows, 512:])
            maskbs = []
            for b in range(BPC):
                maskb = sm.tile([P, SRC], F32, tag="maskb", name=f"maskb{b}")
                nc.sync.dma_start(out=maskb[:],
                                  in_=mask_d[b].to_broadcast((P, SRC)))
                maskb_hf = sm.tile([P, SRC], FP16, tag="maskb_hf",
                                   name=f"maskb_hf{b}")
                nc.vector.tensor_copy(maskb_hf[:], maskb[:])
                maskbs.append((maskb, maskb_hf))
            encT_sb = []
            val_sb = []
            for b in range(BPC):
                e_t = big.tile([P, nE, SRC], FP16, tag="encT", bufs=2,
                               name=f"encT_sb{b}")
                for g in range(2):
                    gs = slice(g * (nE // 2), (g + 1) * (nE // 2))
                    nc.sync.dma_start(out=e_t[:, gs, :],
                                      in_=encT_d[b, :, gs, :])
                v_t = big.tile([P, nS, TRGD], FP16, tag="val", bufs=2,
                               name=f"val_sb{b}")
                for g in range(2):
                    gs = slice(g * (nS // 2), (g + 1) * (nS // 2))
                    nc.sync.dma_start(out=v_t[:, gs, :],
                                      in_=val_d[b, :, gs, :])
                encT_sb.append(e_t)
                val_sb.append(v_t)

            # ---- mm1: HpT[e, t01] = sum_d W[d,e] * hidT01[d, t01] ----
            # two half-passes of 4 et each (4 psum bufs per pass).  dt-outer
            # for DMA pair-wise consumption, but the last two dt rounds go
            # et-wise with the drain right after each stop so ring slots free
            # up staggered instead of all at the end.
            HpT = big.tile([P, nE, TRG2], FP16, tag="HpT", name="HpT")
            drain_eng = [0]

            def drain(dst, src):
                if drain_eng[0] % 2 == 0:
                    nc.vector.tensor_copy(dst, src)
                else:
                    nc.scalar.copy(dst, src)
                drain_eng[0] += 1

            def mm1_mm(pp, dt, et):
                for h in range(2):
                    hs = slice(h * 512, (h + 1) * 512)
                    chain(nc.tensor.matmul(
                        pp[:, hs], w_sb[dt][:, et * P:(et + 1) * P],
                        hidT_sb[dt][:, hs],
                        start=(dt == 0), stop=(dt == nD - 1)))

            def emit_mm1_pass(ets, warm=False):
                pps = [psp.tile([P, TRG2], F32, tag="ps", name=f"mm1ps{et}")
                       for et in ets]
                if warm:
                    # junk transposes of the identity: keep the PE busy
                    # during the DMA/preamble dead time so the HAM clock
                    # gate is warm (2.4GHz) when real work arrives.  The
                    # garbage psum is overwritten by mm1's start=True.
                    junk_view = pps[0][:].bitcast(FP16)
                    for _ in range(24):
                        chain(nc.tensor.matmul(
                            junk_view[:, 0:P], identh[:], identh[:],
                            is_transpose=True, skip_group_check=True))
                for dt in range(nD - 2):
                    for i, et in enumerate(ets):
                        mm1_mm(pps[i], dt, et)
                for i, et in enumerate(ets):
                    for dt in (nD - 2, nD - 1):
                        mm1_mm(pps[i], dt, et)
                    drain(HpT[:, et, :], pps[i][:])

            emit_mm1_pass(range(0, nE // 2), warm=True)
            emit_mm1_pass(range(nE // 2, nE))

            # ---- mm2 + masked softmax over 8 supertiles (b, tt) ----
            tiles = [(b, tt) for b in range(BPC) for tt in range(nT)]
            pks = []
            attnTs = {}

            def emit_mm2(k):
                b, tt = tiles[k]
                ts = slice(b * TRG + tt * P, b * TRG + (tt + 1) * P)
                en_ps = psp.tile([P, SRC], F32, tag="ps", name=f"en{b}{tt}")
                for et in range(nE):
                    for h in range(2):
                        hs = slice(h * 512, (h + 1) * 512)
                        chain(nc.tensor.matmul(en_ps[:, hs], HpT[:, et, ts],
                                               encT_sb[b][:, et, hs],
                                               start=(et == 0),
                                               stop=(et == nE - 1)))
                return en_ps

            def emit_softmax(k, en_ps):
                b, tt = tiles[k]
                maskb, maskb_hf = maskbs[b]
                x = sm.tile([P, SRC], F32, tag="x")
                nc.vector.tensor_mul(x[:], en_ps[:], maskb[:])
                # packed [ae | attn] tile: one output DMA per supertile
                pk = sm.tile([P, 2 * SRC], FP16, tag="aeaw", bufs=7,
                             name=f"aeaw{b}{tt}")
                nc.scalar.copy(pk[:, :SRC], x[:])
                negm = sm.tile([P, 1], F32, tag="negm")
                nc.vector.tensor_reduce(negm[:], x[:], axis=AXL.X,
                                        op=ALU.max, negate=True)
                ex = sm.tile([P, SRC], FP16, tag="ex")
                nc.scalar.activation(ex[:], x[:], ACT_EXP, bias=negm[:],
                                     scale=1.0)
                rowsum = sm.tile([P, 1], F32, tag="rowsum")
                nc.vector.scalar_tensor_tensor(ex[:], ex[:], 1.0,
                                               maskb_hf[:],
                                               op0=ALU.mult, op1=ALU.mult,
                                               accum_out=rowsum[:])
                z = sm.tile([P, 1], F32, tag="z")
                nc.vector.tensor_scalar_add(z[:], rowsum[:], 1e-6)
                rz = sm.tile([P, 1], F32, tag="rz")
                nc.vector.reciprocal(rz[:], z[:])
                nc.vector.tensor_scalar_mul(pk[:, SRC:], ex[:], rz[:])
                nc.sync.dma_start(out=aeaw_d[b, tt * P:(tt + 1) * P, :],
                                  in_=pk[:])
                pks.append(pk)

            def emit_tr(k):
                # PE transposes: the DMA-xbar alternative measures ~5-6us
                # per [128,1024] tile on hardware and serializes — PE does
                # all 8 in ~0.9us
                attn = pks[k][:, SRC:]
                trp = psp.tile([P, SRC], F32, tag="ps", name=f"tr{k}")
                trh = trp[:].bitcast(FP16)
                for st in range(nS):
                    chain(nc.tensor.transpose(trh[:, st * P:(st + 1) * P],
                                              attn[:, st * P:(st + 1) * P],
                                              identh[:]))
                attnT = sm.tile([P, nS, P], FP16, tag="attnT",
                                name=f"attnT{k}")
                nc.scalar.copy(attnT[:], trh[:, :SRC])
                attnTs[k] = attnT

            for k in range(len(tiles)):
                if k == len(tiles) - 1:
                    # tr(T0) goes BEFORE mm2(T7) on the PE: its attnT copy
                    # (which also sits ahead of T7's softmax in the Scalar
                    # FIFO) then completes under mm2(T7)'s 3.5us, so mm3(T0)
                    # starts with zero gap at the B->C boundary
                    emit_tr(0)
                en_ps = emit_mm2(k)
                emit_softmax(k, en_ps)

            def emit_mm3(k):
                b, tt = tiles[k]
                attnT = attnTs.pop(k)
                last = (k == len(tiles) - 1)
                ctx_ps = psp.tile([P, TRGD], F32, tag="ps", name=f"ctx{k}")
                for st in range(nS):
                    for h in range(2):
                        hs = slice(h * 512, (h + 1) * 512)
                        chain(nc.tensor.matmul(ctx_ps[:, hs],
                                               attnT[:, st, :],
                                               val_sb[b][:, st, hs],
                                               start=(st == 0),
                                               stop=(st == nS - 1)))
                ctx_sb = sm.tile([P, TRGD], FP16, tag="ctx_sb")
                rows = slice(tt * P, (tt + 1) * P)
                if last:
                    # split the final drain across two engines + two DMAs
                    # issued from two DGE front-ends to shorten the
                    # end-of-kernel tail
                    nc.vector.tensor_copy(ctx_sb[:, :512], ctx_ps[:, :512])
                    nc.scalar.copy(ctx_sb[:, 512:], ctx_ps[:, 512:])
                    nc.sync.dma_start(out=ctx_d[b, rows, 0:512],
                                      in_=ctx_sb[:, :512])
                    nc.scalar.dma_start(out=ctx_d[b, rows, 512:],
                                        in_=ctx_sb[:, 512:])
                else:
                    # DVE, not Scalar: the Scalar FIFO still holds late
                    # attn-transpose issues during early mm3 tiles
                    nc.vector.tensor_copy(ctx_sb[:], ctx_ps[:])
                    nc.sync.dma_start(out=ctx_d[b, rows, :], in_=ctx_sb[:])

            for k in range(len(tiles)):
                if k + 1 < len(tiles):
                    emit_tr(k + 1)
                emit_mm3(k)

    nc.compile()
    return nc


def kernel(hidden, encoder_outputs, encoder_value, encoder_mask, W):
    global LAST_EXEC_NS, LAST_RESULTS
    from concourse.bass_utils import run_bass_kernel_spmd

    if "nc" not in _cache:
        _cache["nc"] = _build()
    nc = _cache["nc"]

    hidden = np.ascontiguousarray(hidden, dtype=np.float32)
    encoder_outputs = np.ascontiguousarray(encoder_outputs, dtype=np.float32)
    encoder_value = np.ascontiguousarray(encoder_value, dtype=np.float32)
    encoder_mask = np.ascontiguousarray(encoder_mask, dtype=np.float32)
    W = np.ascontiguousarray(W, dtype=np.float32)

    w_hf = W.astype(np.float16)
    in_maps = []
    for c in range(NCORES):
        sl = slice(c * BPC, (c + 1) * BPC)
        hid2 = hidden[sl]  # (2, TRG, TRGD)
        hidT01 = np.concatenate([hid2[0].T, hid2[1].T], axis=1)
        # partition-major marshaling: x_d[b, p, tile, free] so each batch
        # loads with 2 contiguous DMAs
        encT = encoder_outputs[sl].transpose(0, 2, 1)  # (BPC, ENCD, SRC)
        encT_pm = encT.reshape(BPC, nE, P, SRC).transpose(0, 2, 1, 3)
        val_pm = encoder_value[sl].reshape(BPC, nS, P, TRGD).transpose(
            0, 2, 1, 3)
        in_maps.append({
            "hidT": np.ascontiguousarray(hidT01.astype(np.float16)),
            "w": w_hf,
            "encT": np.ascontiguousarray(encT_pm.astype(np.float16)),
            "val": np.ascontiguousarray(val_pm.astype(np.float16)),
            "mask": encoder_mask[sl][:, None, :],
        })

    trace = bool(int(os.environ.get("KERNEL_TRACE", "0")))
    res = run_bass_kernel_spmd(nc, in_maps, core_ids=list(range(NCORES)),
                               trace=trace)
    LAST_EXEC_NS = res.exec_time_ns
    LAST_RESULTS = res

    aeaw = [res.results[c]["aeaw"] for c in range(NCORES)]
    context = np.concatenate([res.results[c]["ctx"] for c in range(NCORES)],
                             axis=0).astype(np.float32)
    attn_energies = np.concatenate(
        [a[:, :, :SRC] for a in aeaw], axis=0).astype(np.float32)
    attn_weights = np.concatenate(
        [a[:, :, SRC:] for a in aeaw], axis=0).astype(np.float32)
    return context, attn_weights, attn_energies
